# revision 30
# baseline (speedup 1.0000x reference)
"""Trainium2 Bass kernel for the DistancePositionOperator.

Reference computation (B=2, L=1024, D=128):
    delta[b,i,j,:] = X[b,i,:] - X[b,j,:]
    alpha[i,j]     = 1 / (1 + |i-j|)            (zero on the diagonal)
    d[b,i,j]       = sum_d |delta|              (pairwise L1 distance)
    C[b,i,j]       = alpha[i,j] / (1 + d[b,i,j])
    O[b,i,:]       = sum_j C[b,i,j] * delta[b,i,j,:]
                   = rowsum(C)[b,i] * X[b,i,:] - (C @ X)[b,i,:]

d and C are symmetric in (i,j), so only one of each 128x128 block pair
is computed: with L split into 8 strip-blocks that is 36 blocks per
batch, 72 total -> 9 per core.  Core q (batch q//4, q%4 -> rotation)
computes blocks (I, (I+K) mod 8) for K in 0..3 at I in {q, q+4} plus the
(q, q+4) anti-diagonal block.  Host-side each core's inputs are rotated
by 128*q tokens so every core runs the IDENTICAL program: strip 0
against key span [0,640) and strip 4 against [512,1024), both
contiguous.  The host un-rotates and sums the per-core partial outputs.

Per query row i the elementwise engines compute Abs_i[d, jspan] in bf16:
ACT via Abs activation with per-partition bias, DVE via the standard
tensor_scalar (x - c) abs_max 0 (supports the fast DVE perf modes).
The PE reduces over d by using Abs_i as matmul *weights* against a ones
vector, landing dT[j, i] directly in PSUM in the transposed layout
needed downstream.  C^T = alpha^T/(1+dT) then drives output matmuls
(C^T as weights, [X | 1] as moving) which produce C@X and rowsum(C) in
one pass; the mirror contribution uses the PE-transposed C block.
"""

import numpy as np
import ml_dtypes

B, L, D = 2, 1024, 128
NBLK = L // 128                      # 8 strip blocks per batch
N_CORES = 8
# per-strip i -> engine split (ACT, DVE): ACT Abs ~718ns/row (640w), DVE
# relu/min native tensor_scalar pair ~592ns/row (2x perf mode).
STRIP_SPLIT = {0: (61, 67), 4: (63, 65)}

# program-relative schedule (identical on every core):
#   (query strip, [key blocks])
SCHED = [(0, [0, 1, 2, 3, 4]), (4, [4, 5, 6, 7])]
NBLOCKS = 9

_COMPILED = None


def _engine_pattern(n_act, n_dve):
    counts = [n_act, n_dve]
    acc = [0.0, 0.0]
    pat = []
    for _ in range(sum(counts)):
        for e in range(2):
            acc[e] += counts[e]
        e = max(range(2), key=lambda k: acc[k])
        acc[e] -= sum(counts)
        pat.append("AV"[e])
    return pat


def _build(iters=1):
    """Build + compile the (core-uniform) Bass program."""
    import concourse.bacc as bacc
    import concourse.tile as tile
    import concourse.mybir as mybir
    from concourse.masks import make_identity

    F32, BF16 = mybir.dt.float32, mybir.dt.bfloat16
    AF = mybir.ActivationFunctionType
    ALU = mybir.AluOpType

    nc = bacc.Bacc("TRN2", target_bir_lowering=False, debug=False,
                   num_devices=N_CORES)
    xt16_ap = nc.dram_tensor("xt16", [D, L], BF16, kind="ExternalInput").ap()
    xt32_ap = nc.dram_tensor("xt32b", [D, 256], F32, kind="ExternalInput").ap()
    xaug_ap = nc.dram_tensor("xaug", [NBLK, 128, D + 1], F32,
                             kind="ExternalInput").ap()
    alpha_ap = nc.dram_tensor("alphat", [NBLOCKS, 128, 128], F32,
                              kind="ExternalInput").ap()
    pout_ap = nc.dram_tensor("pout", [NBLK, 128, D], F32,
                             kind="ExternalOutput").ap()

    with tile.TileContext(nc) as tc:
        with tc.tile_pool(name="consts", bufs=1) as consts, \
             tc.tile_pool(name="abs", bufs=20) as abs_pool, \
             tc.tile_pool(name="work", bufs=3) as work, \
             tc.tile_pool(name="dtaps", bufs=2, space="PSUM") as dtaps, \
             tc.tile_pool(name="dtbps", bufs=1, space="PSUM") as dtbps, \
             tc.tile_pool(name="pops", bufs=2, space="PSUM") as pops, \
             tc.tile_pool(name="po2ps", bufs=2, space="PSUM") as po2ps, \
             tc.tile_pool(name="tps", bufs=1, space="PSUM") as tps:

            xt16 = consts.tile([D, L], BF16, tag="xt16")
            xt32 = consts.tile([D, 256], F32, tag="xt32")
            # parallel startup: spread the critical key-span DMAs over
            # several engine queues (SP / Pool / PE run distinct DMA rings)
            nc.sync.dma_start(xt32[:, 0:128], xt32_ap[:, 0:128])
            nc.sync.dma_start(xt16[:, 0:320], xt16_ap[:, 0:320])
            nc.gpsimd.dma_start(xt16[:, 320:640], xt16_ap[:, 320:640])
            nc.scalar.dma_start(xt16[:, 640:L], xt16_ap[:, 640:L])
            nc.scalar.dma_start(xt32[:, 128:256], xt32_ap[:, 128:256])
            xaug = consts.tile([128, NBLK * (D + 1)], F32, tag="xaug")
            for s in range(NBLK):
                eng = nc.gpsimd if s % 2 == 0 else nc.sync
                eng.dma_start(xaug[:, s * (D + 1):(s + 1) * (D + 1)],
                              xaug_ap[s])
            alpha = consts.tile([128, NBLOCKS * 128], F32, tag="alpha")
            for k in range(NBLOCKS):
                eng = nc.gpsimd if k % 2 == 0 else nc.sync
                eng.dma_start(alpha[:, k * 128:(k + 1) * 128], alpha_ap[k])
            ones16 = consts.tile([D, 1], BF16, tag="ones")
            nc.vector.memset(ones16[:], 1.0)
            nones16 = consts.tile([D, 1], BF16, tag="nones")
            nc.vector.memset(nones16[:], -1.0)
            ones128 = consts.tile([128, 128], BF16, tag="ones128")
            nc.vector.memset(ones128[:], 1.0)
            ident16 = consts.tile([128, 128], BF16, tag="ident16")
            make_identity(nc, ident16[:])
            ident = consts.tile([128, 128], F32, tag="ident")
            make_identity(nc, ident[:])

            import contextlib
            loop_cm = (tc.For_i(0, iters, 1) if iters > 1
                       else contextlib.nullcontext())
            with loop_cm:
                _kernel_body(nc, tc, mybir, xt16, xt32, xaug, alpha,
                             ones16, nones16, ones128, ident16, ident,
                             consts, abs_pool, work, dtaps, dtbps, pops,
                             po2ps, tps, pout_ap)

    nc.compile()
    return nc


def _kernel_body(nc, tc, mybir, xt16, xt32, xaug, alpha, ones16, nones16,
                 ones128, ident16, ident, consts, abs_pool, work, dtaps,
                 dtbps, pops, po2ps, tps, pout_ap):
    F32, BF16 = mybir.dt.float32, mybir.dt.bfloat16
    AF = mybir.ActivationFunctionType
    ALU = mybir.AluOpType

    def xaug_blk(s):
        return xaug[:, s * (D + 1):(s + 1) * (D + 1)]

    def finalize(s, acc):
        # O_s = rowsum * X_s - (C@X)_s, straight from the PSUM accumulator
        o = work.tile([128, D], F32, tag="fin")
        nc.vector.scalar_tensor_tensor(
            o[:], xaug_blk(s)[:, 0:D], acc[:, D:D + 1],
            acc[:, 0:D], ALU.mult, ALU.subtract)
        nc.sync.dma_start(pout_ap[s], o[:])

    state = {}

    def emit_rows(R, jblocks, lo, hi):
        nb = len(jblocks)
        jlo = jblocks[0] * 128
        w = nb * 128
        if lo == 0:
            state[R, "dta"] = dtaps.tile([128, 512], F32, tag="dta",
                                         name=f"dta{R}")
            if nb == 5:
                state[R, "dtb"] = dtbps.tile([128, 128], F32, tag="dtb",
                                             name=f"dtb{R}")
        dta = state[R, "dta"]
        dtb = state.get((R, "dtb"))

        def dt_col(k, i):
            if k < 4:
                return dta[:, k * 128 + i:k * 128 + i + 1]
            return dtb[:, i:i + 1]

        pat = _engine_pattern(*STRIP_SPLIT[R])
        boff = 0 if R == 0 else 128
        for i in range(lo, hi):
            gi = boff + i
            if pat[i] == "A":
                ab = abs_pool.tile([D, w], BF16, tag="ab", name=f"ab{R}_{i}")
                nc.scalar.activation(
                    ab[:], xt16[:, jlo:jlo + w], AF.Abs,
                    bias=xt32[:, gi:gi + 1], scale=-1.0)
                for k in range(nb):
                    nc.tensor.matmul(
                        dt_col(k, i),
                        lhsT=ab[:, k * 128:(k + 1) * 128],
                        rhs=ones16[:], start=True, stop=True)
            else:
                pp = abs_pool.tile([D, w], BF16, tag="ab", name=f"pp{R}_{i}")
                nc.vector.tensor_scalar(
                    pp[:], xt16[:, jlo:jlo + w], xt32[:, gi:gi + 1],
                    0.0, ALU.subtract, ALU.max)
                mm = abs_pool.tile([D, w], BF16, tag="ab", name=f"mm{R}_{i}")
                nc.vector.tensor_scalar(
                    mm[:], xt16[:, jlo:jlo + w], xt32[:, gi:gi + 1],
                    0.0, ALU.subtract, ALU.min)
                for k in range(nb):
                    nc.tensor.matmul(
                        dt_col(k, i),
                        lhsT=pp[:, k * 128:(k + 1) * 128],
                        rhs=ones16[:], start=True, stop=False)
                    nc.tensor.matmul(
                        dt_col(k, i),
                        lhsT=mm[:, k * 128:(k + 1) * 128],
                        rhs=nones16[:], start=False, stop=True)

    def emit_downstream(R, jblocks, blk0, tail):
        # tail=True: ACT is idle, use it for u/ctT to pipeline with DVE.
        # tail=False (mid-stream): keep ACT pure-rows; u/ctT go on DVE,
        # whose deps are met by emission placement so it doesn't stall.
        nb = len(jblocks)
        dta = state[R, "dta"]
        dtb = state.get((R, "dtb"))

        def dt_blk(k):
            if k < 4:
                return dta[:, k * 128:(k + 1) * 128]
            return dtb[:]

        def emit_u(dst, src):
            if tail:
                nc.scalar.add(dst, src, 1.0)
            else:
                nc.vector.tensor_scalar_add(dst, src, 1.0)

        def emit_ctT(dst, src):
            if tail:
                nc.scalar.copy(dst, src)
            else:
                nc.vector.tensor_scalar_add(dst, src, 0.0)

        if R == 0:
            po = pops.tile([128, D + 1], F32, tag="po", name="po0")
            po_started = False
        else:
            po = state["po4"]
            po_started = True

        blk = blk0
        for k, J in enumerate(jblocks):
            u = work.tile([128, 128], F32, tag="u")
            emit_u(u[:], dt_blk(k))
            r = work.tile([128, 128], F32, tag="r")
            nc.vector.reciprocal_approx_fast(r[:], u[:])
            ct = work.tile([128, 128], F32, tag="ct")
            nc.gpsimd.tensor_tensor(
                ct[:], r[:], alpha[:, blk * 128:(blk + 1) * 128],
                ALU.mult)
            # O_R partial: accumulate [C@X | rowsum] over this strip's blocks
            nc.tensor.matmul(po[:], lhsT=ct[:], rhs=xaug_blk(J),
                             start=not po_started, stop=(k == nb - 1),
                             skip_group_check=True)
            po_started = True
            if J != R:
                pt = tps.tile([128, 128], F32, tag="pt")
                nc.tensor.transpose(pt[:], ct[:], ident[:])
                ctT = work.tile([128, 128], F32, tag="ctT")
                emit_ctT(ctT[:], pt[:])
                if R == 0 and J == 4:
                    # opens strip 4's accumulation group
                    po4 = pops.tile([128, D + 1], F32, tag="po", name="po4")
                    state["po4"] = po4
                    nc.tensor.matmul(po4[:], lhsT=ctT[:], rhs=xaug_blk(R),
                                     start=True, stop=False,
                                     skip_group_check=True)
                else:
                    po2 = po2ps.tile([128, D + 1], F32, tag="po2")
                    nc.tensor.matmul(po2[:], lhsT=ctT[:], rhs=xaug_blk(R),
                                     start=True, stop=True)
                    finalize(J, po2[:])
            blk += 1
        finalize(R, po[:])

    (R0, jb0), (R4, jb4) = SCHED
    emit_rows(R0, jb0, 0, 128)
    # strip-4 rows stream immediately; strip-0 downstream is emitted a few
    # rows in, so its dependencies are met by the time the queues reach it
    emit_rows(R4, jb4, 0, 24)
    emit_downstream(R0, jb0, 0, tail=False)
    emit_rows(R4, jb4, 24, 128)
    emit_downstream(R4, jb4, len(jb0), tail=True)


_ALPHA_CACHE = {}


def _core_alpha(q):
    if q in _ALPHA_CACHE:
        return _ALPHA_CACHE[q]
    idx = np.arange(L, dtype=np.float64)
    rot = 128 * q
    real = (idx + rot) % L
    al = np.empty((NBLOCKS, 128, 128), dtype=np.float32)
    k = 0
    for R, jblocks in SCHED:
        ti = real[R * 128:(R + 1) * 128]
        for J in jblocks:
            tj = real[J * 128:(J + 1) * 128]
            dist = np.abs(tj[:, None] - ti[None, :])
            a = 1.0 / (1.0 + dist)
            a[dist == 0] = 0.0
            al[k] = a.astype(np.float32)
            k += 1
    _ALPHA_CACHE[q] = al
    return al


def _prep_host(X):
    """Per-core rotated input dicts. X: [B, L, D] float32."""
    in_maps = []
    for c in range(N_CORES):
        b, q = c // 4, c % 4
        rot = 128 * q
        Xr = np.roll(X[b], -rot, axis=0)          # program token t = real t+rot
        xtT = np.ascontiguousarray(Xr.T)          # [D, L]
        xt16 = xtT.astype(ml_dtypes.bfloat16)
        xt32b = np.ascontiguousarray(
            np.concatenate([xtT[:, 0:128], xtT[:, 512:640]], axis=1))
        xaug = np.concatenate(
            [Xr, np.ones((L, 1), dtype=np.float32)], axis=1)
        xaug = np.ascontiguousarray(xaug.reshape(NBLK, 128, D + 1))
        in_maps.append({"xt16": xt16, "xt32b": xt32b, "xaug": xaug,
                        "alphat": _core_alpha(q)})
    return in_maps


def _get_compiled():
    global _COMPILED
    if _COMPILED is None:
        _COMPILED = _build()
    return _COMPILED


def kernel(X, _trace=False, _trace_kwargs=None):
    """X: np.ndarray [2, 1024, 128] float32 -> O [2, 1024, 128] float32."""
    from concourse.bass_utils import run_bass_kernel_spmd

    X = np.asarray(X, dtype=np.float32)
    assert X.shape == (B, L, D)
    nc = _get_compiled()
    in_maps = _prep_host(X)
    res = run_bass_kernel_spmd(nc, in_maps, list(range(N_CORES)),
                               trace=_trace, **(_trace_kwargs or {}))
    O = np.zeros((B, L, D), dtype=np.float32)
    for c in range(N_CORES):
        b, q = c // 4, c % 4
        part = res.results[c]["pout"].reshape(L, D)
        O[b] += np.roll(part, 128 * q, axis=0)    # un-rotate
    if _trace:
        return O, res
    return O


if __name__ == "__main__":
    rng = np.random.default_rng(0)
    X = rng.standard_normal((B, L, D), dtype=np.float32)
    O = kernel(X)
    print("ok", O.shape, float(np.abs(O).max()))


# revision 31
# speedup vs baseline: 1.0314x; 1.0314x over previous
"""Trainium2 Bass kernel for the DistancePositionOperator.

Reference computation (B=2, L=1024, D=128):
    delta[b,i,j,:] = X[b,i,:] - X[b,j,:]
    alpha[i,j]     = 1 / (1 + |i-j|)            (zero on the diagonal)
    d[b,i,j]       = sum_d |delta|              (pairwise L1 distance)
    C[b,i,j]       = alpha[i,j] / (1 + d[b,i,j])
    O[b,i,:]       = sum_j C[b,i,j] * delta[b,i,j,:]
                   = rowsum(C)[b,i] * X[b,i,:] - (C @ X)[b,i,:]

d and C are symmetric in (i,j), so only one of each 128x128 block pair
is computed: with L split into 8 strip-blocks that is 36 blocks per
batch, 72 total -> 9 per core.  Core q (batch q//4, q%4 -> rotation)
computes blocks (I, (I+K) mod 8) for K in 0..3 at I in {q, q+4} plus the
(q, q+4) anti-diagonal block.  Host-side each core's inputs are rotated
by 128*q tokens so every core runs the IDENTICAL program: strip 0
against key span [0,640) and strip 4 against [512,1024), both
contiguous.  The host un-rotates and sums the per-core partial outputs.

Per query row i the elementwise engines compute Abs_i[d, jspan] in bf16:
ACT via Abs activation with per-partition bias, DVE via the standard
tensor_scalar (x - c) abs_max 0 (supports the fast DVE perf modes).
The PE reduces over d by using Abs_i as matmul *weights* against a ones
vector, landing dT[j, i] directly in PSUM in the transposed layout
needed downstream.  C^T = alpha^T/(1+dT) then drives output matmuls
(C^T as weights, [X | 1] as moving) which produce C@X and rowsum(C) in
one pass; the mirror contribution uses the PE-transposed C block.
"""

import numpy as np
import ml_dtypes

B, L, D = 2, 1024, 128
NBLK = L // 128                      # 8 strip blocks per batch
N_CORES = 8
# per-strip i -> engine split (ACT, DVE): ACT Abs ~718ns/row (640w), DVE
# relu/min native tensor_scalar pair ~592ns/row (2x perf mode).
STRIP_SPLIT = {0: (58, 70), 4: (59, 69)}

# program-relative schedule (identical on every core):
#   (query strip, [key blocks])
SCHED = [(0, [0, 1, 2, 3, 4]), (4, [4, 5, 6, 7])]
NBLOCKS = 9

_COMPILED = None


def _engine_pattern(n_act, n_dve):
    counts = [n_act, n_dve]
    acc = [0.0, 0.0]
    pat = []
    for _ in range(sum(counts)):
        for e in range(2):
            acc[e] += counts[e]
        e = max(range(2), key=lambda k: acc[k])
        acc[e] -= sum(counts)
        pat.append("AV"[e])
    return pat


def _build(iters=1):
    """Build + compile the (core-uniform) Bass program."""
    import concourse.bacc as bacc
    import concourse.tile as tile
    import concourse.mybir as mybir
    from concourse.masks import make_identity

    F32, BF16 = mybir.dt.float32, mybir.dt.bfloat16
    AF = mybir.ActivationFunctionType
    ALU = mybir.AluOpType

    nc = bacc.Bacc("TRN2", target_bir_lowering=False, debug=False,
                   num_devices=N_CORES)
    xt16_ap = nc.dram_tensor("xt16", [D, L], BF16, kind="ExternalInput").ap()
    xt32_ap = nc.dram_tensor("xt32b", [D, 256], F32, kind="ExternalInput").ap()
    xaug_ap = nc.dram_tensor("xaug", [NBLK, 128, D + 1], F32,
                             kind="ExternalInput").ap()
    alpha_ap = nc.dram_tensor("alphat", [NBLOCKS, 128, 128], F32,
                              kind="ExternalInput").ap()
    pout_ap = nc.dram_tensor("pout", [NBLK, 128, D], F32,
                             kind="ExternalOutput").ap()

    with tile.TileContext(nc) as tc:
        with tc.tile_pool(name="consts", bufs=1) as consts, \
             tc.tile_pool(name="abs", bufs=20) as abs_pool, \
             tc.tile_pool(name="work", bufs=3) as work, \
             tc.tile_pool(name="dtaps", bufs=2, space="PSUM") as dtaps, \
             tc.tile_pool(name="dtbps", bufs=1, space="PSUM") as dtbps, \
             tc.tile_pool(name="pops", bufs=2, space="PSUM") as pops, \
             tc.tile_pool(name="po2ps", bufs=2, space="PSUM") as po2ps, \
             tc.tile_pool(name="tps", bufs=1, space="PSUM") as tps:

            xt16 = consts.tile([D, L], BF16, tag="xt16")
            xt32 = consts.tile([D, 256], F32, tag="xt32")
            # parallel startup: spread the critical key-span DMAs over
            # several engine queues (SP / Pool / PE run distinct DMA rings)
            nc.sync.dma_start(xt32[:, 0:128], xt32_ap[:, 0:128])
            nc.sync.dma_start(xt16[:, 0:320], xt16_ap[:, 0:320])
            nc.gpsimd.dma_start(xt16[:, 320:640], xt16_ap[:, 320:640])
            nc.scalar.dma_start(xt16[:, 640:L], xt16_ap[:, 640:L])
            nc.scalar.dma_start(xt32[:, 128:256], xt32_ap[:, 128:256])
            xaug = consts.tile([128, NBLK * (D + 1)], F32, tag="xaug")
            for s in range(NBLK):
                eng = nc.gpsimd if s % 2 == 0 else nc.sync
                eng.dma_start(xaug[:, s * (D + 1):(s + 1) * (D + 1)],
                              xaug_ap[s])
            alpha = consts.tile([128, NBLOCKS * 128], F32, tag="alpha")
            for k in range(NBLOCKS):
                eng = nc.gpsimd if k % 2 == 0 else nc.sync
                eng.dma_start(alpha[:, k * 128:(k + 1) * 128], alpha_ap[k])
            ones16 = consts.tile([D, 1], BF16, tag="ones")
            nc.vector.memset(ones16[:], 1.0)
            nones16 = consts.tile([D, 1], BF16, tag="nones")
            nc.vector.memset(nones16[:], -1.0)
            ones128 = consts.tile([128, 128], BF16, tag="ones128")
            nc.vector.memset(ones128[:], 1.0)
            ident16 = consts.tile([128, 128], BF16, tag="ident16")
            make_identity(nc, ident16[:])
            ident = consts.tile([128, 128], F32, tag="ident")
            make_identity(nc, ident[:])

            import contextlib
            loop_cm = (tc.For_i(0, iters, 1) if iters > 1
                       else contextlib.nullcontext())
            with loop_cm:
                _kernel_body(nc, tc, mybir, xt16, xt32, xaug, alpha,
                             ones16, nones16, ones128, ident16, ident,
                             consts, abs_pool, work, dtaps, dtbps, pops,
                             po2ps, tps, pout_ap)

    nc.compile()
    return nc


def _kernel_body(nc, tc, mybir, xt16, xt32, xaug, alpha, ones16, nones16,
                 ones128, ident16, ident, consts, abs_pool, work, dtaps,
                 dtbps, pops, po2ps, tps, pout_ap):
    F32, BF16 = mybir.dt.float32, mybir.dt.bfloat16
    AF = mybir.ActivationFunctionType
    ALU = mybir.AluOpType

    def xaug_blk(s):
        return xaug[:, s * (D + 1):(s + 1) * (D + 1)]

    def finalize(s, acc):
        # O_s = rowsum * X_s - (C@X)_s, straight from the PSUM accumulator
        o = work.tile([128, D], F32, tag="fin")
        nc.vector.scalar_tensor_tensor(
            o[:], xaug_blk(s)[:, 0:D], acc[:, D:D + 1],
            acc[:, 0:D], ALU.mult, ALU.subtract)
        nc.sync.dma_start(pout_ap[s], o[:])

    state = {}

    def emit_rows(R, jblocks, lo, hi):
        nb = len(jblocks)
        jlo = jblocks[0] * 128
        w = nb * 128
        if lo == 0:
            state[R, "dta"] = dtaps.tile([128, 512], F32, tag="dta",
                                         name=f"dta{R}")
            if nb == 5:
                state[R, "dtb"] = dtbps.tile([128, 128], F32, tag="dtb",
                                             name=f"dtb{R}")
        dta = state[R, "dta"]
        dtb = state.get((R, "dtb"))

        def dt_col(k, i):
            if k < 4:
                return dta[:, k * 128 + i:k * 128 + i + 1]
            return dtb[:, i:i + 1]

        pat = _engine_pattern(*STRIP_SPLIT[R])
        boff = 0 if R == 0 else 128
        for i in range(lo, hi):
            gi = boff + i
            if pat[i] == "A":
                ab = abs_pool.tile([D, w], BF16, tag="ab", name=f"ab{R}_{i}")
                nc.scalar.activation(
                    ab[:], xt16[:, jlo:jlo + w], AF.Abs,
                    bias=xt32[:, gi:gi + 1], scale=-1.0)
                for k in range(nb):
                    nc.tensor.matmul(
                        dt_col(k, i),
                        lhsT=ab[:, k * 128:(k + 1) * 128],
                        rhs=ones16[:], start=True, stop=True)
            else:
                pp = abs_pool.tile([D, w], BF16, tag="ab", name=f"pp{R}_{i}")
                nc.vector.tensor_scalar(
                    pp[:], xt16[:, jlo:jlo + w], xt32[:, gi:gi + 1],
                    0.0, ALU.subtract, ALU.max)
                mm = abs_pool.tile([D, w], BF16, tag="ab", name=f"mm{R}_{i}")
                nc.vector.tensor_scalar(
                    mm[:], xt16[:, jlo:jlo + w], xt32[:, gi:gi + 1],
                    0.0, ALU.subtract, ALU.min)
                for k in range(nb):
                    nc.tensor.matmul(
                        dt_col(k, i),
                        lhsT=pp[:, k * 128:(k + 1) * 128],
                        rhs=ones16[:], start=True, stop=False)
                    nc.tensor.matmul(
                        dt_col(k, i),
                        lhsT=mm[:, k * 128:(k + 1) * 128],
                        rhs=nones16[:], start=False, stop=True)

    def emit_downstream(R, jblocks, blk0, tail):
        # tail=True: ACT is idle, use it for u/ctT to pipeline with DVE.
        # tail=False (mid-stream): keep ACT pure-rows; u/ctT go on DVE,
        # whose deps are met by emission placement so it doesn't stall.
        nb = len(jblocks)
        dta = state[R, "dta"]
        dtb = state.get((R, "dtb"))

        def dt_blk(k):
            if k < 4:
                return dta[:, k * 128:(k + 1) * 128]
            return dtb[:]

        def emit_u(dst, src):
            if tail:
                nc.scalar.add(dst, src, 1.0)
            else:
                nc.vector.tensor_scalar_add(dst, src, 1.0)

        def emit_ctT(dst, src):
            if tail:
                nc.scalar.copy(dst, src)
            else:
                nc.vector.tensor_scalar_add(dst, src, 0.0)

        if R == 0:
            po = pops.tile([128, D + 1], F32, tag="po", name="po0")
            po_started = False
        else:
            po = state["po4"]
            po_started = True

        blk = blk0
        for k, J in enumerate(jblocks):
            u = work.tile([128, 128], F32, tag="u")
            emit_u(u[:], dt_blk(k))
            r = work.tile([128, 128], F32, tag="r")
            nc.vector.reciprocal_approx_fast(r[:], u[:])
            ct = work.tile([128, 128], F32, tag="ct")
            nc.gpsimd.tensor_tensor(
                ct[:], r[:], alpha[:, blk * 128:(blk + 1) * 128],
                ALU.mult)
            # O_R partial: accumulate [C@X | rowsum] over this strip's blocks
            nc.tensor.matmul(po[:], lhsT=ct[:], rhs=xaug_blk(J),
                             start=not po_started, stop=(k == nb - 1),
                             skip_group_check=True)
            po_started = True
            if J != R:
                pt = tps.tile([128, 128], F32, tag="pt")
                nc.tensor.transpose(pt[:], ct[:], ident[:])
                ctT = work.tile([128, 128], F32, tag="ctT")
                emit_ctT(ctT[:], pt[:])
                if R == 0 and J == 4:
                    # opens strip 4's accumulation group
                    po4 = pops.tile([128, D + 1], F32, tag="po", name="po4")
                    state["po4"] = po4
                    nc.tensor.matmul(po4[:], lhsT=ctT[:], rhs=xaug_blk(R),
                                     start=True, stop=False,
                                     skip_group_check=True)
                else:
                    po2 = po2ps.tile([128, D + 1], F32, tag="po2")
                    nc.tensor.matmul(po2[:], lhsT=ctT[:], rhs=xaug_blk(R),
                                     start=True, stop=True)
                    finalize(J, po2[:])
            blk += 1
        finalize(R, po[:])

    (R0, jb0), (R4, jb4) = SCHED
    emit_rows(R0, jb0, 0, 128)
    # strip-4 rows stream immediately; strip-0 downstream is emitted a few
    # rows in, so its dependencies are met by the time the queues reach it
    emit_rows(R4, jb4, 0, 40)
    emit_downstream(R0, jb0, 0, tail=False)
    emit_rows(R4, jb4, 40, 128)
    emit_downstream(R4, jb4, len(jb0), tail=True)


_ALPHA_CACHE = {}


def _core_alpha(q):
    if q in _ALPHA_CACHE:
        return _ALPHA_CACHE[q]
    idx = np.arange(L, dtype=np.float64)
    rot = 128 * q
    real = (idx + rot) % L
    al = np.empty((NBLOCKS, 128, 128), dtype=np.float32)
    k = 0
    for R, jblocks in SCHED:
        ti = real[R * 128:(R + 1) * 128]
        for J in jblocks:
            tj = real[J * 128:(J + 1) * 128]
            dist = np.abs(tj[:, None] - ti[None, :])
            a = 1.0 / (1.0 + dist)
            a[dist == 0] = 0.0
            al[k] = a.astype(np.float32)
            k += 1
    _ALPHA_CACHE[q] = al
    return al


def _prep_host(X):
    """Per-core rotated input dicts. X: [B, L, D] float32."""
    in_maps = []
    for c in range(N_CORES):
        b, q = c // 4, c % 4
        rot = 128 * q
        Xr = np.roll(X[b], -rot, axis=0)          # program token t = real t+rot
        xtT = np.ascontiguousarray(Xr.T)          # [D, L]
        xt16 = xtT.astype(ml_dtypes.bfloat16)
        xt32b = np.ascontiguousarray(
            np.concatenate([xtT[:, 0:128], xtT[:, 512:640]], axis=1))
        xaug = np.concatenate(
            [Xr, np.ones((L, 1), dtype=np.float32)], axis=1)
        xaug = np.ascontiguousarray(xaug.reshape(NBLK, 128, D + 1))
        in_maps.append({"xt16": xt16, "xt32b": xt32b, "xaug": xaug,
                        "alphat": _core_alpha(q)})
    return in_maps


def _get_compiled():
    global _COMPILED
    if _COMPILED is None:
        _COMPILED = _build()
    return _COMPILED


def kernel(X, _trace=False, _trace_kwargs=None):
    """X: np.ndarray [2, 1024, 128] float32 -> O [2, 1024, 128] float32."""
    from concourse.bass_utils import run_bass_kernel_spmd

    X = np.asarray(X, dtype=np.float32)
    assert X.shape == (B, L, D)
    nc = _get_compiled()
    in_maps = _prep_host(X)
    res = run_bass_kernel_spmd(nc, in_maps, list(range(N_CORES)),
                               trace=_trace, **(_trace_kwargs or {}))
    O = np.zeros((B, L, D), dtype=np.float32)
    for c in range(N_CORES):
        b, q = c // 4, c % 4
        part = res.results[c]["pout"].reshape(L, D)
        O[b] += np.roll(part, 128 * q, axis=0)    # un-rotate
    if _trace:
        return O, res
    return O


if __name__ == "__main__":
    rng = np.random.default_rng(0)
    X = rng.standard_normal((B, L, D), dtype=np.float32)
    O = kernel(X)
    print("ok", O.shape, float(np.abs(O).max()))


# revision 32
# speedup vs baseline: 1.2650x; 1.2265x over previous
"""Trainium2 Bass kernel for the DistancePositionOperator.

Reference computation (B=2, L=1024, D=128):
    delta[b,i,j,:] = X[b,i,:] - X[b,j,:]
    alpha[i,j]     = 1 / (1 + |i-j|)            (zero on the diagonal)
    d[b,i,j]       = sum_d |delta|              (pairwise L1 distance)
    C[b,i,j]       = alpha[i,j] / (1 + d[b,i,j])
    O[b,i,:]       = sum_j C[b,i,j] * delta[b,i,j,:]
                   = rowsum(C)[b,i] * X[b,i,:] - (C @ X)[b,i,:]

d and C are symmetric in (i,j), so only one of each 128x128 block pair
is computed: with L split into 8 strip-blocks that is 36 blocks per
batch, 72 total -> 9 per core.  Core q (batch q//4, q%4 -> rotation)
computes blocks (I, (I+K) mod 8) for K in 0..3 at I in {q, q+4} plus the
(q, q+4) anti-diagonal block.  Host-side each core's inputs are rotated
by 128*q tokens so every core runs the IDENTICAL program: strip 0
against key span [0,640) and strip 4 against [512,1024), both
contiguous.  The host un-rotates and sums the per-core partial outputs.

Per query row i the elementwise engines compute Abs_i[d, jspan] in bf16:
ACT via Abs activation with per-partition bias, DVE via the standard
tensor_scalar (x - c) abs_max 0 (supports the fast DVE perf modes).
The PE reduces over d by using Abs_i as matmul *weights* against a ones
vector, landing dT[j, i] directly in PSUM in the transposed layout
needed downstream.  C^T = alpha^T/(1+dT) then drives output matmuls
(C^T as weights, [X | 1] as moving) which produce C@X and rowsum(C) in
one pass; the mirror contribution uses the PE-transposed C block.
"""

import numpy as np
import ml_dtypes

B, L, D = 2, 1024, 128
NBLK = L // 128                      # 8 strip blocks per batch
N_CORES = 8
# per-strip i -> engine split (ACT, DVE): ACT Abs ~718ns/row (640w), DVE
# relu/min native tensor_scalar pair ~592ns/row (2x perf mode).
STRIP_SPLIT = {0: (43, 85), 4: (45, 83)}

# program-relative schedule (identical on every core):
#   (query strip, [key blocks])
SCHED = [(0, [0, 1, 2, 3, 4]), (4, [4, 5, 6, 7])]
NBLOCKS = 9

_COMPILED = None


def _engine_pattern(n_act, n_dve):
    counts = [n_act, n_dve]
    acc = [0.0, 0.0]
    pat = []
    for _ in range(sum(counts)):
        for e in range(2):
            acc[e] += counts[e]
        e = max(range(2), key=lambda k: acc[k])
        acc[e] -= sum(counts)
        pat.append("AV"[e])
    return pat


def _build(iters=1):
    """Build + compile the (core-uniform) Bass program."""
    import concourse.bacc as bacc
    import concourse.tile as tile
    import concourse.mybir as mybir
    from concourse.masks import make_identity

    F32, BF16 = mybir.dt.float32, mybir.dt.bfloat16
    AF = mybir.ActivationFunctionType
    ALU = mybir.AluOpType

    nc = bacc.Bacc("TRN2", target_bir_lowering=False, debug=False,
                   num_devices=N_CORES)
    xt16_ap = nc.dram_tensor("xt16", [D, L], BF16, kind="ExternalInput").ap()
    xt32_ap = nc.dram_tensor("xt32b", [D, 256], F32, kind="ExternalInput").ap()
    xt32n_ap = nc.dram_tensor("xt32n", [D, 256], F32,
                              kind="ExternalInput").ap()
    sblk_ap = nc.dram_tensor("sblk", [128, NBLOCKS], F32,
                             kind="ExternalInput").ap()
    s1_ap = nc.dram_tensor("s1", [128, 256], F32, kind="ExternalInput").ap()
    xaug_ap = nc.dram_tensor("xaug", [NBLK, 128, D + 1], F32,
                             kind="ExternalInput").ap()
    alpha_ap = nc.dram_tensor("alphat", [NBLOCKS, 128, 128], F32,
                              kind="ExternalInput").ap()
    pout_ap = nc.dram_tensor("pout", [NBLK, 128, D], F32,
                             kind="ExternalOutput").ap()

    with tile.TileContext(nc) as tc:
        with tc.tile_pool(name="consts", bufs=1) as consts, \
             tc.tile_pool(name="abs", bufs=20) as abs_pool, \
             tc.tile_pool(name="work", bufs=3) as work, \
             tc.tile_pool(name="dtaps", bufs=2, space="PSUM") as dtaps, \
             tc.tile_pool(name="dtbps", bufs=1, space="PSUM") as dtbps, \
             tc.tile_pool(name="pops", bufs=2, space="PSUM") as pops, \
             tc.tile_pool(name="po2ps", bufs=2, space="PSUM") as po2ps, \
             tc.tile_pool(name="tps", bufs=1, space="PSUM") as tps:

            xt16 = consts.tile([D, L], BF16, tag="xt16")
            xt32 = consts.tile([D, 256], F32, tag="xt32")
            # parallel startup: spread the critical key-span DMAs over
            # several engine queues (SP / Pool / PE run distinct DMA rings)
            nc.sync.dma_start(xt32[:, 0:128], xt32_ap[:, 0:128])
            nc.sync.dma_start(xt16[:, 0:320], xt16_ap[:, 0:320])
            nc.gpsimd.dma_start(xt16[:, 320:640], xt16_ap[:, 320:640])
            nc.scalar.dma_start(xt16[:, 640:L], xt16_ap[:, 640:L])
            nc.scalar.dma_start(xt32[:, 128:256], xt32_ap[:, 128:256])
            xt32n = consts.tile([D, 256], F32, tag="xt32n")
            nc.sync.dma_start(xt32n[:, 0:128], xt32n_ap[:, 0:128])
            nc.scalar.dma_start(xt32n[:, 128:256], xt32n_ap[:, 128:256])
            sblk = consts.tile([128, NBLOCKS], F32, tag="sblk")
            nc.gpsimd.dma_start(sblk[:], sblk_ap)
            s1 = consts.tile([128, 256], F32, tag="s1")
            nc.gpsimd.dma_start(s1[:], s1_ap)
            xaug = consts.tile([128, NBLK * (D + 1)], F32, tag="xaug")
            for s in range(NBLK):
                eng = nc.gpsimd if s % 2 == 0 else nc.sync
                eng.dma_start(xaug[:, s * (D + 1):(s + 1) * (D + 1)],
                              xaug_ap[s])
            alpha = consts.tile([128, NBLOCKS * 128], F32, tag="alpha")
            for k in range(NBLOCKS):
                eng = nc.gpsimd if k % 2 == 0 else nc.sync
                eng.dma_start(alpha[:, k * 128:(k + 1) * 128], alpha_ap[k])
            twos16 = consts.tile([D, 1], BF16, tag="twos")
            nc.vector.memset(twos16[:], 2.0)
            ident = consts.tile([128, 128], F32, tag="ident")
            make_identity(nc, ident[:])

            import contextlib
            loop_cm = (tc.For_i(0, iters, 1) if iters > 1
                       else contextlib.nullcontext())
            with loop_cm:
                _kernel_body(nc, tc, mybir, xt16, xt32, xt32n, sblk, s1,
                             xaug, alpha, twos16, ident,
                             consts, abs_pool, work, dtaps, dtbps, pops,
                             po2ps, tps, pout_ap)

    nc.compile()
    return nc


def _kernel_body(nc, tc, mybir, xt16, xt32, xt32n, sblk, s1, xaug, alpha,
                 twos16, ident, consts, abs_pool, work, dtaps,
                 dtbps, pops, po2ps, tps, pout_ap):
    F32, BF16 = mybir.dt.float32, mybir.dt.bfloat16
    AF = mybir.ActivationFunctionType
    ALU = mybir.AluOpType

    def xaug_blk(s):
        return xaug[:, s * (D + 1):(s + 1) * (D + 1)]

    def finalize(s, acc):
        # O_s = rowsum * X_s - (C@X)_s, straight from the PSUM accumulator
        o = work.tile([128, D], F32, tag="fin")
        nc.vector.scalar_tensor_tensor(
            o[:], xaug_blk(s)[:, 0:D], acc[:, D:D + 1],
            acc[:, 0:D], ALU.mult, ALU.subtract)
        nc.sync.dma_start(pout_ap[s], o[:])

    state = {}

    def emit_rows(R, jblocks, lo, hi):
        nb = len(jblocks)
        jlo = jblocks[0] * 128
        w = nb * 128
        if lo == 0:
            state[R, "dta"] = dtaps.tile([128, 512], F32, tag="dta",
                                         name=f"dta{R}")
            if nb == 5:
                state[R, "dtb"] = dtbps.tile([128, 128], F32, tag="dtb",
                                             name=f"dtb{R}")
        dta = state[R, "dta"]
        dtb = state.get((R, "dtb"))

        def dt_col(k, i):
            if k < 4:
                return dta[:, k * 128 + i:k * 128 + i + 1]
            return dtb[:, i:i + 1]

        pat = _engine_pattern(*STRIP_SPLIT[R])
        boff = 0 if R == 0 else 128
        for i in range(lo, hi):
            gi = boff + i
            ab = abs_pool.tile([D, w], BF16, tag="ab", name=f"ab{R}_{i}")
            if pat[i] == "A":
                nc.scalar.activation(
                    ab[:], xt16[:, jlo:jlo + w], AF.Relu,
                    bias=xt32n[:, gi:gi + 1], scale=1.0)
            else:
                nc.vector.tensor_scalar(
                    ab[:], xt16[:, jlo:jlo + w], xt32[:, gi:gi + 1],
                    0.0, ALU.subtract, ALU.max)
            for k in range(nb):
                nc.tensor.matmul(
                    dt_col(k, i),
                    lhsT=ab[:, k * 128:(k + 1) * 128],
                    rhs=twos16[:], start=True, stop=True)

    def emit_downstream(R, jblocks, blk0, tail):
        # tail=True: ACT is idle, use it for u/ctT to pipeline with DVE.
        # tail=False (mid-stream): keep ACT pure-rows; u/ctT go on DVE,
        # whose deps are met by emission placement so it doesn't stall.
        nb = len(jblocks)
        dta = state[R, "dta"]
        dtb = state.get((R, "dtb"))

        def dt_blk(k):
            if k < 4:
                return dta[:, k * 128:(k + 1) * 128]
            return dtb[:]

        boff = 0 if R == 0 else 128

        def emit_u(dst, src, blk):
            nc.vector.scalar_tensor_tensor(
                dst, src, sblk[:, blk:blk + 1],
                s1[:, boff:boff + 128], ALU.subtract, ALU.add)

        def emit_ctT(dst, src):
            if tail:
                nc.scalar.copy(dst, src)
            else:
                nc.vector.tensor_scalar_add(dst, src, 0.0)

        if R == 0:
            po = pops.tile([128, D + 1], F32, tag="po", name="po0")
            po_started = False
        else:
            po = state["po4"]
            po_started = True

        blk = blk0
        for k, J in enumerate(jblocks):
            u = work.tile([128, 128], F32, tag="u")
            emit_u(u[:], dt_blk(k), blk)
            r = work.tile([128, 128], F32, tag="r")
            nc.vector.reciprocal_approx_fast(r[:], u[:])
            ct = work.tile([128, 128], F32, tag="ct")
            nc.gpsimd.tensor_tensor(
                ct[:], r[:], alpha[:, blk * 128:(blk + 1) * 128],
                ALU.mult)
            # O_R partial: accumulate [C@X | rowsum] over this strip's blocks
            nc.tensor.matmul(po[:], lhsT=ct[:], rhs=xaug_blk(J),
                             start=not po_started, stop=(k == nb - 1),
                             skip_group_check=True)
            po_started = True
            if J != R:
                pt = tps.tile([128, 128], F32, tag="pt")
                nc.tensor.transpose(pt[:], ct[:], ident[:])
                ctT = work.tile([128, 128], F32, tag="ctT")
                emit_ctT(ctT[:], pt[:])
                if R == 0 and J == 4:
                    # opens strip 4's accumulation group
                    po4 = pops.tile([128, D + 1], F32, tag="po", name="po4")
                    state["po4"] = po4
                    nc.tensor.matmul(po4[:], lhsT=ctT[:], rhs=xaug_blk(R),
                                     start=True, stop=False,
                                     skip_group_check=True)
                else:
                    po2 = po2ps.tile([128, D + 1], F32, tag="po2")
                    nc.tensor.matmul(po2[:], lhsT=ctT[:], rhs=xaug_blk(R),
                                     start=True, stop=True)
                    finalize(J, po2[:])
            blk += 1
        finalize(R, po[:])

    (R0, jb0), (R4, jb4) = SCHED
    emit_rows(R0, jb0, 0, 128)
    # strip-4 rows stream immediately; strip-0 downstream is emitted a few
    # rows in, so its dependencies are met by the time the queues reach it
    emit_rows(R4, jb4, 0, 40)
    emit_downstream(R0, jb0, 0, tail=False)
    emit_rows(R4, jb4, 40, 128)
    emit_downstream(R4, jb4, len(jb0), tail=True)


_ALPHA_CACHE = {}


def _core_alpha(q):
    if q in _ALPHA_CACHE:
        return _ALPHA_CACHE[q]
    idx = np.arange(L, dtype=np.float64)
    rot = 128 * q
    real = (idx + rot) % L
    al = np.empty((NBLOCKS, 128, 128), dtype=np.float32)
    k = 0
    for R, jblocks in SCHED:
        ti = real[R * 128:(R + 1) * 128]
        for J in jblocks:
            tj = real[J * 128:(J + 1) * 128]
            dist = np.abs(tj[:, None] - ti[None, :])
            a = 1.0 / (1.0 + dist)
            a[dist == 0] = 0.0
            al[k] = a.astype(np.float32)
            k += 1
    _ALPHA_CACHE[q] = al
    return al


def _prep_host(X):
    """Per-core rotated input dicts. X: [B, L, D] float32."""
    in_maps = []
    for c in range(N_CORES):
        b, q = c // 4, c % 4
        rot = 128 * q
        Xr = np.roll(X[b], -rot, axis=0)          # program token t = real t+rot
        xtT = np.ascontiguousarray(Xr.T)          # [D, L]
        xt16 = xtT.astype(ml_dtypes.bfloat16)
        xt32b = np.ascontiguousarray(
            np.concatenate([xtT[:, 0:128], xtT[:, 512:640]], axis=1))
        xt32n = np.ascontiguousarray(-xt32b)
        S = xt16.astype(np.float32).sum(axis=0)          # [L] from bf16 X^T
        sblk = np.empty((128, NBLOCKS), dtype=np.float32)
        kk = 0
        for R, jblocks in SCHED:
            for J in jblocks:
                sblk[:, kk] = S[J * 128:(J + 1) * 128]
                kk += 1
        s1 = np.empty((128, 256), dtype=np.float32)
        s1[:, 0:128] = 1.0 + S[0:128][None, :]
        s1[:, 128:256] = 1.0 + S[512:640][None, :]
        xaug = np.concatenate(
            [Xr, np.ones((L, 1), dtype=np.float32)], axis=1)
        xaug = np.ascontiguousarray(xaug.reshape(NBLK, 128, D + 1))
        in_maps.append({"xt16": xt16, "xt32b": xt32b, "xt32n": xt32n,
                        "sblk": sblk, "s1": s1, "xaug": xaug,
                        "alphat": _core_alpha(q)})
    return in_maps


def _get_compiled():
    global _COMPILED
    if _COMPILED is None:
        _COMPILED = _build()
    return _COMPILED


def kernel(X, _trace=False, _trace_kwargs=None):
    """X: np.ndarray [2, 1024, 128] float32 -> O [2, 1024, 128] float32."""
    from concourse.bass_utils import run_bass_kernel_spmd

    X = np.asarray(X, dtype=np.float32)
    assert X.shape == (B, L, D)
    nc = _get_compiled()
    in_maps = _prep_host(X)
    res = run_bass_kernel_spmd(nc, in_maps, list(range(N_CORES)),
                               trace=_trace, **(_trace_kwargs or {}))
    O = np.zeros((B, L, D), dtype=np.float32)
    for c in range(N_CORES):
        b, q = c // 4, c % 4
        part = res.results[c]["pout"].reshape(L, D)
        O[b] += np.roll(part, 128 * q, axis=0)    # un-rotate
    if _trace:
        return O, res
    return O


if __name__ == "__main__":
    rng = np.random.default_rng(0)
    X = rng.standard_normal((B, L, D), dtype=np.float32)
    O = kernel(X)
    print("ok", O.shape, float(np.abs(O).max()))


# revision 33
# speedup vs baseline: 1.3319x; 1.0529x over previous
"""Trainium2 Bass kernel for the DistancePositionOperator.

Reference computation (B=2, L=1024, D=128):
    delta[b,i,j,:] = X[b,i,:] - X[b,j,:]
    alpha[i,j]     = 1 / (1 + |i-j|)            (zero on the diagonal)
    d[b,i,j]       = sum_d |delta|              (pairwise L1 distance)
    C[b,i,j]       = alpha[i,j] / (1 + d[b,i,j])
    O[b,i,:]       = sum_j C[b,i,j] * delta[b,i,j,:]
                   = rowsum(C)[b,i] * X[b,i,:] - (C @ X)[b,i,:]

d and C are symmetric in (i,j), so only one of each 128x128 block pair
is computed: with L split into 8 strip-blocks that is 36 blocks per
batch, 72 total -> 9 per core.  Core q (batch q//4, q%4 -> rotation)
computes blocks (I, (I+K) mod 8) for K in 0..3 at I in {q, q+4} plus the
(q, q+4) anti-diagonal block.  Host-side each core's inputs are rotated
by 128*q tokens so every core runs the IDENTICAL program: strip 0
against key span [0,640) and strip 4 against [512,1024), both
contiguous.  The host un-rotates and sums the per-core partial outputs.

Per query row i the elementwise engines compute Abs_i[d, jspan] in bf16:
ACT via Abs activation with per-partition bias, DVE via the standard
tensor_scalar (x - c) abs_max 0 (supports the fast DVE perf modes).
The PE reduces over d by using Abs_i as matmul *weights* against a ones
vector, landing dT[j, i] directly in PSUM in the transposed layout
needed downstream.  C^T = alpha^T/(1+dT) then drives output matmuls
(C^T as weights, [X | 1] as moving) which produce C@X and rowsum(C) in
one pass; the mirror contribution uses the PE-transposed C block.
"""

import numpy as np
import ml_dtypes

B, L, D = 2, 1024, 128
NBLK = L // 128                      # 8 strip blocks per batch
N_CORES = 8
# per-strip i -> engine split (ACT, DVE): ACT Abs ~718ns/row (640w), DVE
# relu/min native tensor_scalar pair ~592ns/row (2x perf mode).
STRIP_SPLIT = {0: (37, 91), 4: (45, 83)}

# program-relative schedule (identical on every core):
#   (query strip, [key blocks])
SCHED = [(0, [0, 1, 2, 3, 4]), (4, [4, 5, 6, 7])]
NBLOCKS = 9

_COMPILED = None


def _engine_pattern(n_act, n_dve):
    counts = [n_act, n_dve]
    acc = [0.0, 0.0]
    pat = []
    for _ in range(sum(counts)):
        for e in range(2):
            acc[e] += counts[e]
        e = max(range(2), key=lambda k: acc[k])
        acc[e] -= sum(counts)
        pat.append("AV"[e])
    return pat


def _build(iters=1):
    """Build + compile the (core-uniform) Bass program."""
    import concourse.bacc as bacc
    import concourse.tile as tile
    import concourse.mybir as mybir
    from concourse.masks import make_identity

    F32, BF16 = mybir.dt.float32, mybir.dt.bfloat16
    AF = mybir.ActivationFunctionType
    ALU = mybir.AluOpType

    nc = bacc.Bacc("TRN2", target_bir_lowering=False, debug=False,
                   num_devices=N_CORES)
    xt16_ap = nc.dram_tensor("xt16", [D, L], BF16, kind="ExternalInput").ap()
    xt32_ap = nc.dram_tensor("xt32b", [D, 256], F32, kind="ExternalInput").ap()
    xt32n_ap = nc.dram_tensor("xt32n", [D, 256], F32,
                              kind="ExternalInput").ap()
    sblk_ap = nc.dram_tensor("sblk", [128, NBLOCKS], F32,
                             kind="ExternalInput").ap()
    s1_ap = nc.dram_tensor("s1", [128, 256], F32, kind="ExternalInput").ap()
    xaug_ap = nc.dram_tensor("xaug", [NBLK, 128, D + 1], F32,
                             kind="ExternalInput").ap()
    alpha_ap = nc.dram_tensor("alphat", [NBLOCKS, 128, 128], F32,
                              kind="ExternalInput").ap()
    pout_ap = nc.dram_tensor("pout", [NBLK, 128, D], F32,
                             kind="ExternalOutput").ap()

    with tile.TileContext(nc) as tc:
        with tc.tile_pool(name="consts", bufs=1) as consts, \
             tc.tile_pool(name="abs", bufs=20) as abs_pool, \
             tc.tile_pool(name="work", bufs=3) as work, \
             tc.tile_pool(name="dtaps", bufs=2, space="PSUM") as dtaps, \
             tc.tile_pool(name="dtbps", bufs=1, space="PSUM") as dtbps, \
             tc.tile_pool(name="pops", bufs=2, space="PSUM") as pops, \
             tc.tile_pool(name="po2ps", bufs=2, space="PSUM") as po2ps, \
             tc.tile_pool(name="tps", bufs=1, space="PSUM") as tps:

            xt16 = consts.tile([D, L], BF16, tag="xt16")
            xt32 = consts.tile([D, 256], F32, tag="xt32")
            # parallel startup: spread the critical key-span DMAs over
            # several engine queues (SP / Pool / PE run distinct DMA rings)
            nc.sync.dma_start(xt32[:, 0:128], xt32_ap[:, 0:128])
            nc.sync.dma_start(xt16[:, 0:320], xt16_ap[:, 0:320])
            nc.gpsimd.dma_start(xt16[:, 320:640], xt16_ap[:, 320:640])
            xt32n = consts.tile([D, 256], F32, tag="xt32n")
            nc.scalar.dma_start(xt32n[:, 0:128], xt32n_ap[:, 0:128])
            nc.scalar.dma_start(xt16[:, 640:L], xt16_ap[:, 640:L])
            nc.scalar.dma_start(xt32[:, 128:256], xt32_ap[:, 128:256])
            nc.scalar.dma_start(xt32n[:, 128:256], xt32n_ap[:, 128:256])
            sblk = consts.tile([128, NBLOCKS], F32, tag="sblk")
            nc.gpsimd.dma_start(sblk[:], sblk_ap)
            s1 = consts.tile([128, 256], F32, tag="s1")
            nc.gpsimd.dma_start(s1[:], s1_ap)
            xaug = consts.tile([128, NBLK * (D + 1)], F32, tag="xaug")
            for s in range(NBLK):
                eng = nc.gpsimd if s % 2 == 0 else nc.sync
                eng.dma_start(xaug[:, s * (D + 1):(s + 1) * (D + 1)],
                              xaug_ap[s])
            alpha = consts.tile([128, NBLOCKS * 128], F32, tag="alpha")
            for k in range(NBLOCKS):
                eng = nc.gpsimd if k % 2 == 0 else nc.sync
                eng.dma_start(alpha[:, k * 128:(k + 1) * 128], alpha_ap[k])
            twos16 = consts.tile([D, 1], BF16, tag="twos")
            nc.vector.memset(twos16[:], 2.0)
            ident = consts.tile([128, 128], F32, tag="ident")
            make_identity(nc, ident[:])

            import contextlib
            loop_cm = (tc.For_i(0, iters, 1) if iters > 1
                       else contextlib.nullcontext())
            with loop_cm:
                _kernel_body(nc, tc, mybir, xt16, xt32, xt32n, sblk, s1,
                             xaug, alpha, twos16, ident,
                             consts, abs_pool, work, dtaps, dtbps, pops,
                             po2ps, tps, pout_ap)

    nc.compile()
    return nc


def _kernel_body(nc, tc, mybir, xt16, xt32, xt32n, sblk, s1, xaug, alpha,
                 twos16, ident, consts, abs_pool, work, dtaps,
                 dtbps, pops, po2ps, tps, pout_ap):
    F32, BF16 = mybir.dt.float32, mybir.dt.bfloat16
    AF = mybir.ActivationFunctionType
    ALU = mybir.AluOpType

    def xaug_blk(s):
        return xaug[:, s * (D + 1):(s + 1) * (D + 1)]

    def finalize(s, acc):
        # O_s = rowsum * X_s - (C@X)_s, straight from the PSUM accumulator
        o = work.tile([128, D], F32, tag="fin")
        nc.vector.scalar_tensor_tensor(
            o[:], xaug_blk(s)[:, 0:D], acc[:, D:D + 1],
            acc[:, 0:D], ALU.mult, ALU.subtract)
        nc.sync.dma_start(pout_ap[s], o[:])

    state = {}

    def emit_rows(R, jblocks, lo, hi):
        nb = len(jblocks)
        jlo = jblocks[0] * 128
        w = nb * 128
        if lo == 0:
            state[R, "dta"] = dtaps.tile([128, 512], F32, tag="dta",
                                         name=f"dta{R}")
            if nb == 5:
                state[R, "dtb"] = dtbps.tile([128, 128], F32, tag="dtb",
                                             name=f"dtb{R}")
        dta = state[R, "dta"]
        dtb = state.get((R, "dtb"))

        def dt_col(k, i):
            if k < 4:
                return dta[:, k * 128 + i:k * 128 + i + 1]
            return dtb[:, i:i + 1]

        pat = _engine_pattern(*STRIP_SPLIT[R])
        boff = 0 if R == 0 else 128
        for i in range(lo, hi):
            gi = boff + i
            ab = abs_pool.tile([D, w], BF16, tag="ab", name=f"ab{R}_{i}")
            if pat[i] == "A":
                nc.scalar.activation(
                    ab[:], xt16[:, jlo:jlo + w], AF.Relu,
                    bias=xt32n[:, gi:gi + 1], scale=1.0)
            else:
                nc.vector.tensor_scalar(
                    ab[:], xt16[:, jlo:jlo + w], xt32[:, gi:gi + 1],
                    0.0, ALU.subtract, ALU.max)
            for k in range(nb):
                nc.tensor.matmul(
                    dt_col(k, i),
                    lhsT=ab[:, k * 128:(k + 1) * 128],
                    rhs=twos16[:], start=True, stop=True)

    def emit_downstream(R, jblocks, blk0, tail):
        # tail=True: ACT is idle, use it for u/ctT to pipeline with DVE.
        # tail=False (mid-stream): keep ACT pure-rows; u/ctT go on DVE,
        # whose deps are met by emission placement so it doesn't stall.
        nb = len(jblocks)
        dta = state[R, "dta"]
        dtb = state.get((R, "dtb"))

        def dt_blk(k):
            if k < 4:
                return dta[:, k * 128:(k + 1) * 128]
            return dtb[:]

        boff = 0 if R == 0 else 128

        def emit_u(dst, src, blk):
            nc.vector.scalar_tensor_tensor(
                dst, src, sblk[:, blk:blk + 1],
                s1[:, boff:boff + 128], ALU.subtract, ALU.add)

        def emit_ctT(dst, src):
            if tail:
                nc.scalar.copy(dst, src)
            else:
                nc.vector.tensor_scalar_add(dst, src, 0.0)

        if R == 0:
            po = pops.tile([128, D + 1], F32, tag="po", name="po0")
            po_started = False
        else:
            po = state["po4"]
            po_started = True

        blk = blk0
        for k, J in enumerate(jblocks):
            u = work.tile([128, 128], F32, tag="u")
            emit_u(u[:], dt_blk(k), blk)
            r = work.tile([128, 128], F32, tag="r")
            nc.vector.reciprocal_approx_fast(r[:], u[:])
            ct = work.tile([128, 128], F32, tag="ct")
            nc.gpsimd.tensor_tensor(
                ct[:], r[:], alpha[:, blk * 128:(blk + 1) * 128],
                ALU.mult)
            # O_R partial: accumulate [C@X | rowsum] over this strip's blocks
            nc.tensor.matmul(po[:], lhsT=ct[:], rhs=xaug_blk(J),
                             start=not po_started, stop=(k == nb - 1),
                             skip_group_check=True)
            po_started = True
            if J != R:
                pt = tps.tile([128, 128], F32, tag="pt")
                nc.tensor.transpose(pt[:], ct[:], ident[:])
                ctT = work.tile([128, 128], F32, tag="ctT")
                emit_ctT(ctT[:], pt[:])
                if R == 0 and J == 4:
                    # opens strip 4's accumulation group
                    po4 = pops.tile([128, D + 1], F32, tag="po", name="po4")
                    state["po4"] = po4
                    nc.tensor.matmul(po4[:], lhsT=ctT[:], rhs=xaug_blk(R),
                                     start=True, stop=False,
                                     skip_group_check=True)
                else:
                    po2 = po2ps.tile([128, D + 1], F32, tag="po2")
                    nc.tensor.matmul(po2[:], lhsT=ctT[:], rhs=xaug_blk(R),
                                     start=True, stop=True)
                    finalize(J, po2[:])
            blk += 1
        finalize(R, po[:])

    (R0, jb0), (R4, jb4) = SCHED
    emit_rows(R0, jb0, 0, 128)
    # strip-4 rows stream immediately; strip-0 downstream is emitted a few
    # rows in, so its dependencies are met by the time the queues reach it
    emit_rows(R4, jb4, 0, 40)
    emit_downstream(R0, jb0, 0, tail=False)
    emit_rows(R4, jb4, 40, 128)
    emit_downstream(R4, jb4, len(jb0), tail=True)


_ALPHA_CACHE = {}


def _core_alpha(q):
    if q in _ALPHA_CACHE:
        return _ALPHA_CACHE[q]
    idx = np.arange(L, dtype=np.float64)
    rot = 128 * q
    real = (idx + rot) % L
    al = np.empty((NBLOCKS, 128, 128), dtype=np.float32)
    k = 0
    for R, jblocks in SCHED:
        ti = real[R * 128:(R + 1) * 128]
        for J in jblocks:
            tj = real[J * 128:(J + 1) * 128]
            dist = np.abs(tj[:, None] - ti[None, :])
            a = 1.0 / (1.0 + dist)
            a[dist == 0] = 0.0
            al[k] = a.astype(np.float32)
            k += 1
    _ALPHA_CACHE[q] = al
    return al


def _prep_host(X):
    """Per-core rotated input dicts. X: [B, L, D] float32."""
    in_maps = []
    for c in range(N_CORES):
        b, q = c // 4, c % 4
        rot = 128 * q
        Xr = np.roll(X[b], -rot, axis=0)          # program token t = real t+rot
        xtT = np.ascontiguousarray(Xr.T)          # [D, L]
        xt16 = xtT.astype(ml_dtypes.bfloat16)
        xt32b = np.ascontiguousarray(
            np.concatenate([xtT[:, 0:128], xtT[:, 512:640]], axis=1))
        xt32n = np.ascontiguousarray(-xt32b)
        S = xt16.astype(np.float32).sum(axis=0)          # [L] from bf16 X^T
        sblk = np.empty((128, NBLOCKS), dtype=np.float32)
        kk = 0
        for R, jblocks in SCHED:
            for J in jblocks:
                sblk[:, kk] = S[J * 128:(J + 1) * 128]
                kk += 1
        s1 = np.empty((128, 256), dtype=np.float32)
        s1[:, 0:128] = 1.0 + S[0:128][None, :]
        s1[:, 128:256] = 1.0 + S[512:640][None, :]
        xaug = np.concatenate(
            [Xr, np.ones((L, 1), dtype=np.float32)], axis=1)
        xaug = np.ascontiguousarray(xaug.reshape(NBLK, 128, D + 1))
        in_maps.append({"xt16": xt16, "xt32b": xt32b, "xt32n": xt32n,
                        "sblk": sblk, "s1": s1, "xaug": xaug,
                        "alphat": _core_alpha(q)})
    return in_maps


def _get_compiled():
    global _COMPILED
    if _COMPILED is None:
        _COMPILED = _build()
    return _COMPILED


def kernel(X, _trace=False, _trace_kwargs=None):
    """X: np.ndarray [2, 1024, 128] float32 -> O [2, 1024, 128] float32."""
    from concourse.bass_utils import run_bass_kernel_spmd

    X = np.asarray(X, dtype=np.float32)
    assert X.shape == (B, L, D)
    nc = _get_compiled()
    in_maps = _prep_host(X)
    res = run_bass_kernel_spmd(nc, in_maps, list(range(N_CORES)),
                               trace=_trace, **(_trace_kwargs or {}))
    O = np.zeros((B, L, D), dtype=np.float32)
    for c in range(N_CORES):
        b, q = c // 4, c % 4
        part = res.results[c]["pout"].reshape(L, D)
        O[b] += np.roll(part, 128 * q, axis=0)    # un-rotate
    if _trace:
        return O, res
    return O


if __name__ == "__main__":
    rng = np.random.default_rng(0)
    X = rng.standard_normal((B, L, D), dtype=np.float32)
    O = kernel(X)
    print("ok", O.shape, float(np.abs(O).max()))
